# revision 1
# baseline (speedup 1.0000x reference)
"""Tied-row (MSA) attention, sharded over 8 TRN2 NeuronCores.

Reference computation (b=1, r=128 MSA rows, n=512, 8 heads x 64):
    q, k, v = x @ Wq, x @ Wk, x @ Wv          per-row projections
    dots[h,i,j] = sum_{r,d} q[r,h,i,d] k[r,h,j,d] * scale / sqrt(num_rows)
    attn = softmax_j(dots)                     shared across rows
    out[r,i] = (sum_j attn[h,i,j] v[r,h,j,d]) @ Wo + bo

Sharding: MSA-row axis r split 16-per-core.  Each core computes its partial
logits (reduction over its local r); partials are summed with one bf16
AllReduce per head-pair, pipelined behind the following pairs' matmuls.

This revision computes the logits TRANSPOSED (dotsT[j,i] via stationary=kT),
so the softmax needs no PE transposes: exp runs directly on the AllReduced
dotsT tiles, the denominator Z[i] comes from a ones-vector matmul over the
partition (j) axis, 1/Z is broadcast across partitions with a rank-1
fp32r matmul, and the normalization is an elementwise bf16 multiply.
attn^T @ v and the output projection are core-local.  x is cast to bf16
before its PE transposes (bf16 transposes run 2x faster than fp32);
the attn@v outputs and Wo are bf16 so the output-projection weight loads
hit the fast (FWL) path.
"""

import numpy as np

import concourse.bacc as bacc
import concourse.bass as bass
import concourse.mybir as mybir
import concourse.tile as tile
from concourse import bass_utils
from concourse.masks import make_identity

CORES = 8
R = 16          # MSA rows per core
N = 512         # sequence length
DIM = 256       # model dim
H = 8           # heads
D = 64          # head dim
HD = H * D      # 512
RN = R * N      # 8192 token-rows per core

F32 = mybir.dt.float32
F32R = mybir.dt.float32r
BF16 = mybir.dt.bfloat16

RG = [list(range(CORES))]


def build_nc(scale: float):
    nc = bacc.Bacc(None, target_bir_lowering=False, debug=False)

    x_ext = nc.declare_dram_parameter("x", [RN, DIM], F32, isOutput=False)
    wq_ext = nc.declare_dram_parameter("wq", [DIM, HD], F32, isOutput=False)
    wk_ext = nc.declare_dram_parameter("wk", [DIM, HD], F32, isOutput=False)
    wv_ext = nc.declare_dram_parameter("wv", [DIM, HD], F32, isOutput=False)
    wo_ext = nc.declare_dram_parameter("wo", [HD, DIM], F32, isOutput=False)
    out_ext = nc.declare_dram_parameter("out", [RN, DIM], F32, isOutput=True)

    # alternate PSUM->SBUF copies between DVE and ScalarE so neither gates
    # PSUM-bank recycling
    _cp = [0]

    def cp(out, in_):
        if _cp[0] % 2 == 0:
            nc.vector.tensor_copy(out, in_)
        else:
            nc.scalar.copy(out, in_)
        _cp[0] += 1

    # all bulk DMAs ride the Sync queue: the GpSimd queue must stay
    # collectives-only (instructions behind a collective_compute wait for
    # the collective to finish), and DMAs on Scalar/Vector would head-block
    # the PSUM evictions those engines perform
    def dma(out, in_):
        nc.sync.dma_start(out=out, in_=in_)

    with tile.TileContext(nc) as tc:
        # ---- DRAM bounce buffers: one AllReduce per head-pair ----
        dram = tc.alloc_tile_pool(name="dram", bufs=1, space="DRAM")
        ar_in = [dram.tile([2 * N, N], BF16, tag=f"ar_in{hp}", name=f"ar_in{hp}") for hp in range(4)]
        wu_in = dram.tile([128, 8], BF16, tag="wu_in", name="wu_in")
        wu_out = dram.tile([128, 8], BF16, tag="wu_out", name="wu_out", addr_space="Shared")
        ar_out = [
            dram.tile([2 * N, N], BF16, tag=f"ar_out{hp}", name=f"ar_out{hp}", addr_space="Shared")
            for hp in range(4)
        ]

        # ---- pools (allocated up front; releases must be LIFO per space) ----
        consts = tc.alloc_tile_pool(name="consts", bufs=1)
        v_pool = tc.alloc_tile_pool(name="v", bufs=R * 4)
        attnT_pool = tc.alloc_tile_pool(name="attnT", bufs=1)
        xT_pool = tc.alloc_tile_pool(name="xT", bufs=1)
        xrow_pool = tc.alloc_tile_pool(name="xrow", bufs=8)
        xbf_pool = tc.alloc_tile_pool(name="xbf", bufs=8)
        wstage = tc.alloc_tile_pool(name="wstage", bufs=2)

        # first x rows prefetch, ahead of everything else on the sync queue
        first_xrs = []
        for c in range(4):
            xr = xrow_pool.tile([128, DIM], F32, tag="xr")
            nc.sync.dma_start(out=xr[:], in_=x_ext[c * 128:(c + 1) * 128, :])
            first_xrs.append(xr)

        # ---- constants; only Wq/Wk are needed early (first projections) —
        # stage them now on two queues; Wv/Wo wait until the x stream is out
        wq_sb = consts.tile([128, 2, HD], BF16, tag="wq")
        wk_sb = consts.tile([128, 2, HD], BF16, tag="wk")
        wv_sb = consts.tile([128, 2, HD], BF16, tag="wv")
        wo_sb = consts.tile([128, 4, DIM], BF16, tag="wo")
        idbf = consts.tile([128, 128], BF16, tag="idbf")
        ones_f = consts.tile([128, 128], F32, tag="ones_f")
        ones_bf = consts.tile([128, 128], BF16, tag="ones_bf")
        for weng, wext, wsb in ((nc.sync, wq_ext, wq_sb), (nc.scalar, wk_ext, wk_sb)):
            wf = wstage.tile([128, 2, HD], F32, tag="wf")
            weng.dma_start(
                out=wf[:], in_=wext[:, :].rearrange("(k p) n -> p k n", p=128)
            )
            nc.any.tensor_copy(wsb[:], wf[:])
        make_identity(nc, idbf[:])
        nc.vector.memset(ones_f[:], 1.0)
        nc.vector.tensor_copy(ones_bf[:], ones_f[:])

        # warm up ncfw so the first real AllReduce skips the cold-start lag
        nc.sync.dma_start(out=wu_in[:, :], in_=idbf[:, 0:8])
        nc.gpsimd.collective_compute(
            "AllReduce",
            mybir.AluOpType.add,
            replica_groups=RG,
            ins=[wu_in[:, :].opt()],
            outs=[wu_out[:, :].opt()],
        )

        attn = attnT_pool.tile([128, H, 4, N], BF16, tag="attn")
        xT = xT_pool.tile([128, 2, RN], BF16, tag="xT")

        proj_psum = tc.alloc_tile_pool(name="proj_psum", bufs=3, space="PSUM")
        dots_psum = tc.alloc_tile_pool(name="dots_psum", bufs=3, space="PSUM")
        xp_psum = tc.alloc_tile_pool(name="xp_psum", bufs=2, space="PSUM")

        # ---- load x, cast to bf16, transpose to x^T [dim(2x128), rn] ----
        # 4 PE transposes batched per PSUM bank -> one [128,512] copy out
        _cast = [0]
        for c4 in range(RN // N):
            xbs = []
            for j in range(4):
                c = c4 * 4 + j
                if c < 4:
                    xr = first_xrs[c]
                else:
                    xr = xrow_pool.tile([128, DIM], F32, tag="xr")
                    # the x load paces the whole kernel front and is
                    # queue-serialized; put 3 of every 8 chunks on the Scalar
                    # queue (idle early) so two DMAs run concurrently
                    if c % 8 >= 5:
                        nc.scalar.dma_start(out=xr[:], in_=x_ext[c * 128:(c + 1) * 128, :])
                    else:
                        nc.sync.dma_start(out=xr[:], in_=x_ext[c * 128:(c + 1) * 128, :])
                xb = xbf_pool.tile([128, DIM], BF16, tag="xb")
                # fp32->bf16 casts: round-robin DVE / ScalarE (GpSimd would
                # sit behind the warmup collective on its queue)
                if _cast[0] % 2 == 0:
                    nc.vector.tensor_copy(xb[:], xr[:])
                else:
                    nc.scalar.copy(xb[:], xr[:])
                _cast[0] += 1
                xbs.append(xb)
            for kc in range(2):
                pt = xp_psum.tile([128, N], BF16, tag="xp")
                for j in range(4):
                    nc.tensor.transpose(
                        pt[:, j * 128:(j + 1) * 128],
                        xbs[j][:, kc * 128:(kc + 1) * 128],
                        idbf[:],
                    )
                cp(xT[:, kc, c4 * N:(c4 + 1) * N], pt[:])

        # late weight staging: Wv (first use ~halfway in) and Wo (last third)
        wvf = wstage.tile([128, 2, HD], F32, tag="wf")
        nc.sync.dma_start(
            out=wvf[:], in_=wv_ext[:, :].rearrange("(k p) n -> p k n", p=128)
        )
        nc.any.tensor_copy(wv_sb[:], wvf[:])
        wof = wstage.tile([128, 4, DIM], F32, tag="wf")
        nc.sync.dma_start(
            out=wof[:], in_=wo_ext[:, :].rearrange("(k p) n -> p k n", p=128)
        )
        nc.any.tensor_copy(wo_sb[:], wof[:])

        wstage.release()
        xbf_pool.release()
        xrow_pool.release()
        xp_psum.release()

        dstage_pool = tc.alloc_tile_pool(name="dstage", bufs=4)
        smax_pool = tc.alloc_tile_pool(name="smax", bufs=2)
        qkT_pool = tc.alloc_tile_pool(name="qkT", bufs=1)

        def softmax_local(hp, spool, zpool, sfx, wait_ms):
            """exp + transpose-free normalize of both heads of AllReduce #hp
            into attn[:, 2hp+m, :, :] (tiles are [j-part, i-free]).

            wait_ms biases the Tile scheduler: this whole chain is gated on
            AllReduce #hp, so model it as not-ready before then — otherwise
            its ops head-block the Sync/Scalar/Vector queues ahead of
            independent work (the scheduler's collective timing is a guess)."""
            with tc.tile_wait_until(wait_ms):
                for m in range(2):
                    h = 2 * hp + m
                    exps = []
                    for jc in range(4):
                        zt = spool.tile([128, N], BF16, tag="zt" + sfx)
                        row0 = m * N + jc * 128
                        dma(zt[:], ar_out[hp][row0:row0 + 128, :])
                        et = spool.tile([128, N], BF16, tag="et" + sfx, bufs=5)
                        nc.scalar.activation(
                            et[:], zt[:], mybir.ActivationFunctionType.Exp, scale=scale
                        )
                        exps.append(et)
                    # Z[i] broadcast to all partitions: all-ones stationary
                    # sums exp over the partition (j) axis into every row
                    bps = zpool.tile([128, N], F32, tag="bps" + sfx)
                    for jc in range(4):
                        nc.tensor.matmul(
                            bps[:],
                            ones_bf[:],
                            exps[jc][:],
                            start=(jc == 0),
                            stop=(jc == 3),
                        )
                    rz = spool.tile([128, N], BF16, tag="rz" + sfx)
                    with nc.allow_low_precision(reason="1/Z scale fine in bf16"):
                        nc.vector.reciprocal(rz[:], bps[:])
                    for jc in range(4):
                        nc.vector.tensor_mul(attn[:, h, jc, :], exps[jc][:], rz[:])

        for hp in range(4):
            qT = qkT_pool.tile([128, RN], BF16, tag="qT")
            kT = qkT_pool.tile([128, RN], BF16, tag="kT")
            for wsb, dstT in ((wq_sb, qT), (wk_sb, kT)):
                for ch in range(RN // N):
                    ps = proj_psum.tile([128, N], F32, tag="proj")
                    for kc in range(2):
                        nc.tensor.matmul(
                            ps[:],
                            wsb[:, kc, hp * 128:(hp + 1) * 128],
                            xT[:, kc, ch * N:(ch + 1) * N],
                            start=(kc == 0),
                            stop=(kc == 1),
                        )
                    cp(dstT[:, ch * N:(ch + 1) * N], ps[:])

            # partial dotsT[j,i] for the two heads of this pair; the even head
            # uses PE row-group 0-63, the odd head 64-127 (concurrent tiles)
            for jc in range(4):
                pe_ = dots_psum.tile([128, N], F32, tag="dots")
                po_ = dots_psum.tile([128, N], F32, tag="dots")
                for rr in range(R):
                    base = rr * N
                    jsl = slice(base + jc * 128, base + jc * 128 + 128)
                    isl = slice(base, base + N)
                    nc.tensor.matmul(
                        pe_[:],
                        kT[0:64, jsl],
                        qT[0:64, isl],
                        start=(rr == 0),
                        stop=(rr == R - 1),
                        skip_group_check=True,
                    )
                    nc.tensor.matmul(
                        po_[:],
                        kT[64:128, jsl],
                        qT[64:128, isl],
                        start=(rr == 0),
                        stop=(rr == R - 1),
                        skip_group_check=True,
                    )
                for m, ps in ((0, pe_), (1, po_)):
                    st = dstage_pool.tile([128, N], BF16, tag="dstage")
                    cp(st[:], ps[:])
                    row0 = m * N + jc * 128
                    dma(ar_in[hp][row0:row0 + 128, :], st[:])

            nc.gpsimd.collective_compute(
                "AllReduce",
                mybir.AluOpType.add,
                replica_groups=RG,
                ins=[ar_in[hp][:, :].opt()],
                outs=[ar_out[hp][:, :].opt()],
            )

        qkT_pool.release()
        z_psum = tc.alloc_tile_pool(name="z_psum", bufs=2, space="PSUM")

        # ---- v projection (overlaps the AllReduces; reads xT) ----
        v_tiles = {}
        for rr in range(R):
            if rr == 3:
                softmax_local(0, smax_pool, z_psum, "", 0.17)
            if rr == 9:
                softmax_local(1, smax_pool, z_psum, "", 0.24)
            for jt in range(4):
                ps = proj_psum.tile([128, N], F32, tag="proj")
                for kc in range(2):
                    nc.tensor.matmul(
                        ps[:],
                        xT[:, kc, rr * N + jt * 128:rr * N + jt * 128 + 128],
                        wv_sb[:, kc, :],
                        start=(kc == 0),
                        stop=(kc == 1),
                    )
                vt = v_pool.tile([128, HD], BF16, tag="v")
                cp(vt[:], ps[:])
                v_tiles[(rr, jt)] = vt

        z_psum.release()
        smax_pool.release()
        dstage_pool.release()
        xT_pool.release()
        dots_psum.release()
        proj_psum.release()

        # ---- attn^T @ v -> out^T (bf16), then out @ Wo ----
        # head-pair-major: all rows of pair 0, then 1, 2, 3 — each softmax's
        # AllReduce hides behind ~2 pairs' worth (~35us) of attn@v compute.
        # The output projection for row r follows pair 3's block for row r.
        oT_pool = tc.alloc_tile_pool(name="oT", bufs=52)
        fstage_pool = tc.alloc_tile_pool(name="fstage", bufs=6)
        av_psum = tc.alloc_tile_pool(name="av_psum", bufs=3, space="PSUM")
        fin_psum = tc.alloc_tile_pool(name="fin_psum", bufs=3, space="PSUM")
        z2_psum = tc.alloc_tile_pool(name="z2_psum", bufs=2, space="PSUM")

        _oq = [0]
        oTs = {}
        for hp in range(4):
            if hp == 2:
                softmax_local(2, fstage_pool, z2_psum, "2", 0.26)
            if hp == 3:
                softmax_local(3, fstage_pool, z2_psum, "2", 0.29)
            for rr in range(R):
                ps = av_psum.tile([128, N], F32, tag="av")
                for jt in range(4):
                    for m in range(2):
                        h = 2 * hp + m
                        nc.tensor.matmul(
                            ps[m * 64:(m + 1) * 64, :],
                            v_tiles[(rr, jt)][:, h * D:(h + 1) * D],
                            attn[:, h, jt, :],
                            start=(jt == 0),
                            stop=(jt == 3),
                            tile_position=(0, m * 64),
                            skip_group_check=True,
                        )
                oT = oT_pool.tile([128, N], BF16, tag="oT")
                cp(oT[:], ps[:])
                oTs[(rr, hp)] = oT
                if hp == 3:
                    for ic in range(4):
                        psf = fin_psum.tile([128, DIM], F32, tag="fin")
                        for kc in range(4):
                            nc.tensor.matmul(
                                psf[:],
                                oTs[(rr, kc)][:, ic * 128:(ic + 1) * 128],
                                wo_sb[:, kc, :],
                                start=(kc == 0),
                                stop=(kc == 3),
                            )
                        fst = fstage_pool.tile([128, DIM], F32, tag="fst")
                        cp(fst[:], psf[:])
                        row0 = rr * N + ic * 128
                        # the stores run well past the last collective: split
                        # them onto the (now idle) GpSimd queue as a 2nd lane
                        if _oq[0] % 2 == 0:
                            nc.gpsimd.dma_start(out=out_ext[row0:row0 + 128, :], in_=fst[:])
                        else:
                            nc.sync.dma_start(out=out_ext[row0:row0 + 128, :], in_=fst[:])
                        _oq[0] += 1

        z2_psum.release()
        fin_psum.release()
        av_psum.release()
        fstage_pool.release()
        oT_pool.release()
        attnT_pool.release()
        v_pool.release()
        consts.release()
        dram.release()

    if not nc.is_finalized():
        nc.finalize()
    return nc


_cache = {}


def _get_nc(scale: float):
    key = round(float(scale), 12)
    if key not in _cache:
        _cache[key] = build_nc(float(scale))
    return _cache[key]


def make_in_maps(x, Wq, Wkv, Wo):
    x = np.ascontiguousarray(np.asarray(x, dtype=np.float32)).reshape(CORES, RN, DIM)
    Wq = np.ascontiguousarray(np.asarray(Wq, dtype=np.float32))
    Wkv = np.asarray(Wkv, dtype=np.float32)
    Wk = np.ascontiguousarray(Wkv[:, :HD])
    Wv = np.ascontiguousarray(Wkv[:, HD:])
    Wo = np.ascontiguousarray(np.asarray(Wo, dtype=np.float32))
    return [
        {"x": x[c], "wq": Wq, "wk": Wk, "wv": Wv, "wo": Wo} for c in range(CORES)
    ]


def kernel(x, Wq, Wkv, Wo, bo, mask, tie_attn_dim):
    x = np.asarray(x)
    br, n, dim = x.shape
    r = int(tie_attn_dim)
    assert (br, n, dim) == (128, 512, 256) and r == 128, "kernel hardcodes shapes"
    mask = np.asarray(mask)
    assert mask.all(), "kernel assumes an all-valid mask"
    num_rows = float(mask.reshape(1, r, n).any(axis=-1).sum(axis=-1)[0])
    scale = (D ** -0.5) * (num_rows ** -0.5)

    nc = _get_nc(scale)
    in_maps = make_in_maps(x, Wq, Wkv, Wo)
    res = bass_utils.run_bass_kernel_spmd(nc, in_maps, core_ids=list(range(CORES)))
    out = np.concatenate([m["out"] for m in res.results], axis=0)
    out = out.reshape(br, n, dim)
    bo = np.asarray(bo, dtype=np.float32)
    if bo.any():
        out = out + bo
    return np.ascontiguousarray(out.astype(np.float32))

